# revision 1
# baseline (speedup 1.0000x reference)
"""Trainium2 Bass kernel for nn_LookupTableLayer (embedding_lookup).

Full-input contract: kernel(**inputs) takes the full unsharded numpy inputs,
shards positions across 8 NeuronCores (batch dim), runs one SPMD NEFF on
cores 0-7, and returns the full [16, 512, 32, 128] f32 output.

Algorithm:
  reference: t = 0.1*(table/max(table)) + fixed_table; gather rows at idx;
  concat(ex, ey) then reshape(...,128,2).sum(-1) == pair-sum of each gathered
  row. So out[..., 0:64] = pairsum(tx)[idx0], out[..., 64:128] = pairsum(ty)[idx1].
  We precompute the pair-summed 64-wide tables on-chip (1 MB each), store them
  to DRAM scratch, and gather 256 B rows with SWDGE indirect DMA, using the
  loaded positions tile directly as the offset list.

  Token t of a chunk maps to gather slot (p, c) = (t // C, t % C), so the
  merged [128, C, 128] tile stores to HBM as one DMA with 16 KB contiguous
  runs per partition.
"""

from contextlib import ExitStack

import numpy as np

import concourse.bacc as bacc
import concourse.bass as bass
import concourse.bass_isa as bass_isa
import concourse.mybir as mybir
import concourse.tile as tile
from concourse.bass_utils import run_bass_kernel_spmd

N_CORES = 8
B, M, R, D = 16, 512, 32, 128
TABLE_LEN = 4106
T = (B // N_CORES) * M * R  # 32768 tokens per core
PAIRS = D // 2  # 64
FLAT_N = TABLE_LEN * D // 128  # 4106 (flat table elems per partition)
PAIR_N = FLAT_N // 2  # 2053
CHUNK = 8192  # tokens per gather
NCHUNK = T // CHUNK  # 4
C = CHUNK // 128  # 64 tokens per partition per chunk

F32 = mybir.dt.float32
I32 = mybir.dt.int32

F32 = mybir.dt.float32
I32 = mybir.dt.int32


def _flat(h, p):
    return h[:].rearrange("a b -> (a b)").rearrange("(p n) -> p n", p=p)


def build_nc():
    nc = bacc.Bacc("TRN2", target_bir_lowering=False, debug=False)
    pos = nc.dram_tensor("positions", [T, 2], I32, kind="ExternalInput")
    fixed = nc.dram_tensor("fixed_table", [TABLE_LEN, D], F32, kind="ExternalInput")
    tx = nc.dram_tensor("table_x", [TABLE_LEN, D], F32, kind="ExternalInput")
    ty = nc.dram_tensor("table_y", [TABLE_LEN, D], F32, kind="ExternalInput")
    out = nc.dram_tensor("out", [T, D], F32, kind="ExternalOutput")
    txp_d = nc.dram_tensor("txp", [TABLE_LEN, PAIRS], F32, kind="Internal")
    typ_d = nc.dram_tensor("typ", [TABLE_LEN, PAIRS], F32, kind="Internal")

    with tile.TileContext(nc) as tc, ExitStack() as ctx:
        with tc.tile_pool(name="prep", bufs=1) as prep:
            # ---- table preproc: txp = 0.1/max(x) * pairsum(x) + pairsum(fixed)
            xt = prep.tile([128, FLAT_N], F32)
            yt = prep.tile([128, FLAT_N], F32)
            ft = prep.tile([128, FLAT_N], F32)
            nc.sync.dma_start(xt[:], _flat(tx, 128))
            nc.sync.dma_start(yt[:], _flat(ty, 128))
            nc.sync.dma_start(ft[:], _flat(fixed, 128))

            fp = prep.tile([128, PAIR_N], F32)
            fr = ft[:].rearrange("p (n two) -> p n two", two=2)
            nc.vector.tensor_add(fp[:], fr[:, :, 0], fr[:, :, 1])

            for src_t, dram in ((xt, txp_d), (yt, typ_d)):
                mx = prep.tile([128, 1], F32, tag="mx")
                nc.vector.reduce_max(mx[:], src_t[:], axis=mybir.AxisListType.X)
                gm = prep.tile([128, 1], F32, tag="gm")
                nc.gpsimd.partition_all_reduce(gm[:], mx[:], 128, bass_isa.ReduceOp.max)
                sc = prep.tile([128, 1], F32, tag="sc")
                nc.vector.reciprocal(sc[:], gm[:])
                nc.vector.tensor_scalar_mul(sc[:], sc[:], 0.1)
                pr = src_t[:].rearrange("p (n two) -> p n two", two=2)
                ps = prep.tile([128, PAIR_N], F32, tag="ps")
                nc.vector.tensor_add(ps[:], pr[:, :, 0], pr[:, :, 1])
                nc.vector.scalar_tensor_tensor(
                    ps[:], ps[:], sc[:, 0:1], fp[:],
                    op0=mybir.AluOpType.mult, op1=mybir.AluOpType.add,
                )
                nc.sync.dma_start(_flat(dram, 128), ps[:])

        # ---- main loop: per chunk, gather via indirect DMA, merge, store
        pp = ctx.enter_context(tc.tile_pool(name="pos", bufs=2))
        gp = ctx.enter_context(tc.tile_pool(name="g", bufs=2))
        mp = ctx.enter_context(tc.tile_pool(name="m", bufs=2))
        for k in range(NCHUNK):
            posc = pp.tile([128, C, 2], I32, tag="posc")
            nc.sync.dma_start(
                posc[:],
                pos[k * CHUNK : (k + 1) * CHUNK, :].rearrange(
                    "(p c) two -> p c two", p=128
                ),
            )
            px = pp.tile([128, C], I32, tag="px")
            py = pp.tile([128, C], I32, tag="py")
            nc.vector.tensor_copy(px[:], posc[:, :, 0])
            nc.vector.tensor_copy(py[:], posc[:, :, 1])
            gx = gp.tile([128, C, PAIRS], F32, tag="gx")
            gy = gp.tile([128, C, PAIRS], F32, tag="gy")
            for c in range(C):
                nc.gpsimd.indirect_dma_start(
                    out=gx[:, c, :],
                    out_offset=None,
                    in_=txp_d[:],
                    in_offset=bass.IndirectOffsetOnAxis(ap=px[:, c : c + 1], axis=0),
                )
                nc.gpsimd.indirect_dma_start(
                    out=gy[:, c, :],
                    out_offset=None,
                    in_=typ_d[:],
                    in_offset=bass.IndirectOffsetOnAxis(ap=py[:, c : c + 1], axis=0),
                )
            mg = mp.tile([128, C, D], F32, tag="mg")
            nc.vector.tensor_copy(mg[:, :, 0:PAIRS], gx[:])
            nc.vector.tensor_copy(mg[:, :, PAIRS:D], gy[:])
            nc.sync.dma_start(
                out[k * CHUNK : (k + 1) * CHUNK, :].rearrange(
                    "(p c) f -> p (c f)", p=128
                ),
                mg[:].rearrange("p c f -> p (c f)"),
            )

    nc.compile()
    return nc


_cache = {}


def kernel(positions, fixed_table, table_x, table_y):
    nc = _cache.get("nc")
    if nc is None:
        nc = _cache["nc"] = build_nc()
    pos_flat = np.ascontiguousarray(positions.reshape(-1, 2))
    shards = np.split(pos_flat, N_CORES, axis=0)
    fixed_table = np.ascontiguousarray(fixed_table, dtype=np.float32)
    table_x = np.ascontiguousarray(table_x, dtype=np.float32)
    table_y = np.ascontiguousarray(table_y, dtype=np.float32)
    in_maps = [
        {
            "positions": np.ascontiguousarray(s),
            "fixed_table": fixed_table,
            "table_x": table_x,
            "table_y": table_y,
        }
        for s in shards
    ]
    res = run_bass_kernel_spmd(nc, in_maps, core_ids=list(range(N_CORES)))
    outs = [r["out"] for r in res.results]
    return np.concatenate(outs, axis=0).reshape(B, M, R, D)



# revision 9
# speedup vs baseline: 1.1903x; 1.1903x over previous
"""Trainium2 Bass kernel for nn_LookupTableLayer (embedding_lookup).

Full-input contract: kernel(**inputs) takes the full unsharded numpy inputs,
shards positions across 8 NeuronCores (batch dim), runs one SPMD NEFF on
cores 0-7, and returns the full [16, 512, 32, 128] f32 output.

Algorithm:
  reference: t = 0.1*(table/max(table)) + fixed_table; gather rows at idx;
  concat(ex, ey) then reshape(...,128,2).sum(-1) == pair-sum of each gathered
  row. So out[..., 0:64] = pairsum(tx)[idx0], out[..., 64:128] = pairsum(ty)[idx1].
  We precompute the pair-summed 64-wide tables on-chip (1 MB each), store them
  to DRAM scratch, then gather 256 B rows with bulk InstDMAGatherAnt (one
  instruction per 8192-token chunk per table — the per-instruction SWDGE
  overhead dominated when issuing one indirect DMA per token column).

  dma_gather semantics: idx element i lives at idxs[i%16, i//16] (16-partition
  wrap, replicated across the 8 gpsimd core groups); the gathered row lands at
  out[i%128, i//128, :]. We load positions so element i of chunk k is token
  t = k*8192 + (i%16)*512 + i//16, giving gather slot (p, c) the token
  q*512 + c*8 + h (p = h*16+q), which stores back to DRAM as 64 runs of
  512 B per partition with stride 4 KB.
"""

from contextlib import ExitStack

import numpy as np

import concourse.bacc as bacc
import concourse.bass as bass
import concourse.bass_isa as bass_isa
import concourse.mybir as mybir
import concourse.tile as tile
from concourse.bass_utils import run_bass_kernel_spmd

N_CORES = 8
B, M, R, D = 16, 512, 32, 128
TABLE_LEN = 4106
T = (B // N_CORES) * M * R  # 32768 tokens per core
PAIRS = D // 2  # 64
FLAT_N = TABLE_LEN * D // 128  # 4106 (flat table elems per partition)
PAIR_N = FLAT_N // 2  # 2053
CHUNK = 8192  # tokens per merge/store tile
NCHUNK = T // CHUNK  # 4
C = CHUNK // 128  # 64 gathered tokens per partition per chunk
C16 = CHUNK // 16  # 512 idx columns per chunk (16-partition wrap)
GL = 1024  # tokens per dma_gather instruction (SWDGE ring holds 1024 descs)
SCRATCH = 16384  # dynamic DMA scratch bytes (ring = SCRATCH/16 descriptors)
NG = CHUNK // GL  # gather slices per chunk
GC = GL // 128  # out columns per gather slice
G16 = GL // 16  # idx columns per gather slice

F32 = mybir.dt.float32
I32 = mybir.dt.int32
I16 = mybir.dt.int16


def _flat(h, p):
    return h[:].rearrange("a b -> (a b)").rearrange("(p n) -> p n", p=p)


def build_nc():
    nc = bacc.Bacc(
        "TRN2",
        target_bir_lowering=False,
        debug=False,
        dynamic_dma_scratch_size=SCRATCH,
    )
    pos = nc.dram_tensor("positions", [T, 2], I32, kind="ExternalInput")
    fixed = nc.dram_tensor("fixed_table", [TABLE_LEN, D], F32, kind="ExternalInput")
    tx = nc.dram_tensor("table_x", [TABLE_LEN, D], F32, kind="ExternalInput")
    ty = nc.dram_tensor("table_y", [TABLE_LEN, D], F32, kind="ExternalInput")
    out = nc.dram_tensor("out", [T, D], F32, kind="ExternalOutput")
    txp_d = nc.dram_tensor("txp", [TABLE_LEN, PAIRS], F32, kind="Internal")
    typ_d = nc.dram_tensor("typ", [TABLE_LEN, PAIRS], F32, kind="Internal")

    with tile.TileContext(nc) as tc, ExitStack() as ctx:
        ip = ctx.enter_context(tc.tile_pool(name="idx", bufs=1))
        px16 = ip.tile([128, NCHUNK, C, 8], I16)
        py16 = ip.tile([128, NCHUNK, C, 8], I16)

        with tc.tile_pool(name="posw", bufs=1) as pw:
            # idx element i of chunk k is read from idxs[i%16, i//16] and its
            # gathered row lands at out[i%128, i//128]. Wrap positions so slot
            # (p, c) holds token p*64 + c: idxs[w, c*8+d] = token index of
            # token (d*16+w)*64 + c, replicated over the 8 gpsimd core groups.
            posw = pw.tile([128, NCHUNK, 8, C, 2], I32)
            src = pos[:].rearrange(
                "(k d w c) j -> w k d c j", k=NCHUNK, d=8, w=16, c=C
            )
            for g in range(8):
                nc.sync.dma_start(posw[16 * g : 16 * (g + 1)], src)
            pw16 = posw[:].bitcast(I16)  # [128, NCHUNK, 8, C, 4]
            nc.vector.tensor_copy(
                px16[:].rearrange("p k c (d one) -> p k c d one", one=1),
                pw16[:, :, :, :, 0:1].rearrange("p k d c one -> p k c d one"),
            )
            nc.vector.tensor_copy(
                py16[:].rearrange("p k c (d one) -> p k c d one", one=1),
                pw16[:, :, :, :, 2:3].rearrange("p k d c one -> p k c d one"),
            )

        with tc.tile_pool(name="prep", bufs=1) as prep:
            # ---- table preproc: txp = 0.1/max(x) * pairsum(x) + pairsum(fixed)
            xt = prep.tile([128, FLAT_N], F32)
            yt = prep.tile([128, FLAT_N], F32)
            ft = prep.tile([128, FLAT_N], F32)
            nc.sync.dma_start(xt[:], _flat(tx, 128))
            nc.sync.dma_start(yt[:], _flat(ty, 128))
            nc.sync.dma_start(ft[:], _flat(fixed, 128))

            fp = prep.tile([128, PAIR_N], F32)
            fr = ft[:].rearrange("p (n two) -> p n two", two=2)
            nc.vector.tensor_add(fp[:], fr[:, :, 0], fr[:, :, 1])

            for src_t, dram in ((xt, txp_d), (yt, typ_d)):
                mx = prep.tile([128, 1], F32, tag="mx")
                nc.vector.reduce_max(mx[:], src_t[:], axis=mybir.AxisListType.X)
                gm = prep.tile([128, 1], F32, tag="gm")
                nc.gpsimd.partition_all_reduce(gm[:], mx[:], 128, bass_isa.ReduceOp.max)
                sc = prep.tile([128, 1], F32, tag="sc")
                nc.vector.reciprocal(sc[:], gm[:])
                nc.vector.tensor_scalar_mul(sc[:], sc[:], 0.1)
                pr = src_t[:].rearrange("p (n two) -> p n two", two=2)
                ps = prep.tile([128, PAIR_N], F32, tag="ps")
                nc.vector.tensor_add(ps[:], pr[:, :, 0], pr[:, :, 1])
                nc.vector.scalar_tensor_tensor(
                    ps[:], ps[:], sc[:, 0:1], fp[:],
                    op0=mybir.AluOpType.mult, op1=mybir.AluOpType.add,
                )
                nc.sync.dma_start(_flat(dram, 128), ps[:])

        # ---- main loop: per chunk, bulk-gather x and y rows, merge, store
        gp = ctx.enter_context(tc.tile_pool(name="g", bufs=2))
        mp = ctx.enter_context(tc.tile_pool(name="m", bufs=2))
        for k in range(NCHUNK):
            gx = gp.tile([128, C, PAIRS], F32, tag="gx")
            gy = gp.tile([128, C, PAIRS], F32, tag="gy")
            # slice the chunk's gather into GL-index instructions: slice j
            # covers idx elements [j*GL, (j+1)*GL) = idx cols [j*G16, ...)
            # and out cols [j*GC, ...). The SWDGE ring caps descriptors per
            # instruction, so one 8192-idx gather is not allowed.
            idxv_x = px16[:, k, :, :].rearrange("p c d -> p (c d)")
            idxv_y = py16[:, k, :, :].rearrange("p c d -> p (c d)")
            for g_t, d_t, idxv in ((gx, txp_d, idxv_x), (gy, typ_d, idxv_y)):
                for j in range(NG):
                    nc.gpsimd.dma_gather(
                        g_t[:, j * GC : (j + 1) * GC, :],
                        d_t[:],
                        idxv[:, j * G16 : (j + 1) * G16],
                        GL,
                        GL,
                        PAIRS,
                    )
            mg = mp.tile([128, C, D], F32, tag="mg")
            nc.vector.tensor_copy(mg[:, :, 0:PAIRS], gx[:])
            nc.scalar.copy(mg[:, :, PAIRS:D], gy[:])
            nc.sync.dma_start(
                out[k * CHUNK : (k + 1) * CHUNK, :].rearrange(
                    "(p c) f -> p (c f)", p=128
                ),
                mg[:].rearrange("p c f -> p (c f)"),
            )

    nc.compile()
    return nc


_cache = {}


def kernel(positions, fixed_table, table_x, table_y):
    nc = _cache.get("nc")
    if nc is None:
        nc = _cache["nc"] = build_nc()
    pos_flat = np.ascontiguousarray(positions.reshape(-1, 2))
    shards = np.split(pos_flat, N_CORES, axis=0)
    fixed_table = np.ascontiguousarray(fixed_table, dtype=np.float32)
    table_x = np.ascontiguousarray(table_x, dtype=np.float32)
    table_y = np.ascontiguousarray(table_y, dtype=np.float32)
    in_maps = [
        {
            "positions": np.ascontiguousarray(s),
            "fixed_table": fixed_table,
            "table_x": table_x,
            "table_y": table_y,
        }
        for s in shards
    ]
    res = run_bass_kernel_spmd(nc, in_maps, core_ids=list(range(N_CORES)))
    outs = [r["out"] for r in res.results]
    return np.concatenate(outs, axis=0).reshape(B, M, R, D)


# revision 11
# speedup vs baseline: 1.2522x; 1.0520x over previous
"""Trainium2 Bass kernel for nn_LookupTableLayer (embedding_lookup).

Full-input contract: kernel(**inputs) takes the full unsharded numpy inputs,
shards positions across 8 NeuronCores (batch dim), runs one SPMD NEFF on
cores 0-7, and returns the full [16, 512, 32, 128] f32 output.

Algorithm:
  reference: t = 0.1*(table/max(table)) + fixed_table; gather rows at idx;
  concat(ex, ey) then reshape(...,128,2).sum(-1) == pair-sum of each gathered
  row. So out[..., 0:64] = pairsum(tx)[idx0], out[..., 64:128] = pairsum(ty)[idx1].
  We precompute the pair-summed 64-wide tables on-chip (1 MB each), store them
  to DRAM scratch, then gather 256 B rows with bulk InstDMAGatherAnt.

  Perf notes (measured on HW):
  - SWDGE descriptor generation on the Pool engine runs at ~8.5 ns/descriptor
    regardless of batching (dma_gather or indirect_dma_start), so the
    2*32768 = 65536 descriptors per core cost ~557 us and dominate. Every
    other engine is scheduled to hide under that stream.
  - dma_gather is capped at 1024 descriptors per instruction (SWDGE ring),
    hence GL=1024 slices.
  - Preproc is ordered so the x pairsum table is in DRAM ~25 us in (ramp),
    and merges/stores are sliced so the post-gather tail is short.

  dma_gather semantics: idx element i of a chunk is read from
  idxs[i%16, i//16] (16-partition wrap, replicated across the 8 gpsimd core
  groups); its gathered row lands at out[i%128, i//128, :]. Positions are
  loaded so slot (p, c) holds token d*1024 + w*64 + c (p = d*16 + w), which
  makes the pos load 512 B-contiguous and the output store 32 KB-contiguous
  per partition.
"""

from contextlib import ExitStack

import numpy as np

import concourse.bacc as bacc
import concourse.bass as bass
import concourse.bass_isa as bass_isa
import concourse.mybir as mybir
import concourse.tile as tile
from concourse.bass_utils import run_bass_kernel_spmd

N_CORES = 8
B, M, R, D = 16, 512, 32, 128
TABLE_LEN = 4106
T = (B // N_CORES) * M * R  # 32768 tokens per core
PAIRS = D // 2  # 64
FLAT_N = TABLE_LEN * D // 128  # 4106 (flat table elems per partition)
PAIR_N = FLAT_N // 2  # 2053
CHUNK = 8192  # tokens per merge/store tile
NCHUNK = T // CHUNK  # 4
C = CHUNK // 128  # 64 gathered tokens per partition per chunk
GL = 1024  # tokens per dma_gather instruction (SWDGE ring = 1024 descs)
NG = CHUNK // GL  # 8 gather slices per chunk
GC = GL // 128  # 8 out columns per gather slice
HALF = C // 2  # store half-chunks to shorten the tail

F32 = mybir.dt.float32
I32 = mybir.dt.int32
I16 = mybir.dt.int16


def _flat(h, p):
    return h[:].rearrange("a b -> (a b)").rearrange("(p n) -> p n", p=p)


def build_nc():
    nc = bacc.Bacc("TRN2", target_bir_lowering=False, debug=False)
    pos = nc.dram_tensor("positions", [T, 2], I32, kind="ExternalInput")
    fixed = nc.dram_tensor("fixed_table", [TABLE_LEN, D], F32, kind="ExternalInput")
    tx = nc.dram_tensor("table_x", [TABLE_LEN, D], F32, kind="ExternalInput")
    ty = nc.dram_tensor("table_y", [TABLE_LEN, D], F32, kind="ExternalInput")
    out = nc.dram_tensor("out", [T, D], F32, kind="ExternalOutput")
    txp_d = nc.dram_tensor("txp", [TABLE_LEN, PAIRS], F32, kind="Internal")
    typ_d = nc.dram_tensor("typ", [TABLE_LEN, PAIRS], F32, kind="Internal")

    with tile.TileContext(nc) as tc, ExitStack() as ctx:
        ip = ctx.enter_context(tc.tile_pool(name="idx", bufs=1))
        px16 = ip.tile([128, NCHUNK, C, 8], I16)
        py16 = ip.tile([128, NCHUNK, C, 8], I16)
        pwp = ctx.enter_context(tc.tile_pool(name="posw", bufs=1))
        posw = pwp.tile([128, NCHUNK, 8, C, 2], I32)

        with tc.tile_pool(name="prep", bufs=1) as prep:
            # ---- table preproc: txp = 0.1/max(x)*pairsum(x) + pairsum(fixed)
            # Ordered so the Pool engine's partition_all_reduces and the txp
            # store land early: the first gather only waits ~25 us.
            xt = prep.tile([128, FLAT_N], F32)
            yt = prep.tile([128, FLAT_N], F32)
            ft = prep.tile([128, FLAT_N], F32)
            nc.sync.dma_start(xt[:], _flat(tx, 128))
            nc.sync.dma_start(yt[:], _flat(ty, 128))
            nc.sync.dma_start(ft[:], _flat(fixed, 128))

            # global max + 0.1/max for both tables first (frees Pool early)
            scs = []
            for src_t in (xt, yt):
                mx = prep.tile([128, 1], F32, tag="mx")
                nc.vector.reduce_max(mx[:], src_t[:], axis=mybir.AxisListType.X)
                gm = prep.tile([128, 1], F32, tag="gm")
                nc.gpsimd.partition_all_reduce(gm[:], mx[:], 128, bass_isa.ReduceOp.max)
                sc = prep.tile([128, 1], F32, tag="sc")
                nc.vector.reciprocal(sc[:], gm[:])
                nc.vector.tensor_scalar_mul(sc[:], sc[:], 0.1)
                scs.append(sc)

            fp = prep.tile([128, PAIR_N], F32)
            fr = ft[:].rearrange("p (n two) -> p n two", two=2)
            nc.vector.tensor_add(fp[:], fr[:, :, 0], fr[:, :, 1])

            for src_t, sc, dram in ((xt, scs[0], txp_d), (yt, scs[1], typ_d)):
                pr = src_t[:].rearrange("p (n two) -> p n two", two=2)
                ps = prep.tile([128, PAIR_N], F32, tag="ps")
                nc.vector.tensor_add(ps[:], pr[:, :, 0], pr[:, :, 1])
                nc.vector.scalar_tensor_tensor(
                    ps[:], ps[:], sc[:, 0:1], fp[:],
                    op0=mybir.AluOpType.mult, op1=mybir.AluOpType.add,
                )
                nc.sync.dma_start(_flat(dram, 128), ps[:])

                if src_t is xt:
                    # idx prep as soon as the x chain is issued: token
                    # k*8192 + d*1024 + w*64 + c lands at posw[16g+w, k, d, c]
                    # for every group g; int16 x/y indices are the low
                    # halfwords of the int32 pairs.
                    src = pos[:].rearrange(
                        "(k d w c) j -> w k d c j", k=NCHUNK, d=8, w=16, c=C
                    )
                    for g in range(8):
                        nc.sync.dma_start(posw[16 * g : 16 * (g + 1)], src)
                    pw16 = posw[:].bitcast(I16)  # [128, k, d, c, 4]
                    nc.vector.tensor_copy(
                        px16[:].rearrange("p k c (d one) -> p k c d one", one=1),
                        pw16[:, :, :, :, 0:1].rearrange(
                            "p k d c one -> p k c d one"
                        ),
                    )
                    nc.vector.tensor_copy(
                        py16[:].rearrange("p k c (d one) -> p k c d one", one=1),
                        pw16[:, :, :, :, 2:3].rearrange(
                            "p k d c one -> p k c d one"
                        ),
                    )

        # ---- main loop: sliced gathers, sliced merges, half-chunk stores
        gp = ctx.enter_context(tc.tile_pool(name="g", bufs=2))
        mp = ctx.enter_context(tc.tile_pool(name="m", bufs=2))
        for k in range(NCHUNK):
            gx = gp.tile([128, C, PAIRS], F32, tag="gx")
            gy = gp.tile([128, C, PAIRS], F32, tag="gy")
            mg = mp.tile([128, C, D], F32, tag="mg")
            idxv_x = px16[:, k, :, :].rearrange("p c d -> p (c d)")
            idxv_y = py16[:, k, :, :].rearrange("p c d -> p (c d)")
            oc = out[k * CHUNK : (k + 1) * CHUNK, :].rearrange(
                "(p c) f -> p c f", p=128
            )
            for j in range(NG):
                cs = slice(j * GC, (j + 1) * GC)
                nc.gpsimd.dma_gather(
                    gx[:, cs, :], txp_d[:],
                    idxv_x[:, j * GL // 16 : (j + 1) * GL // 16],
                    GL, GL, PAIRS,
                )
                nc.gpsimd.dma_gather(
                    gy[:, cs, :], typ_d[:],
                    idxv_y[:, j * GL // 16 : (j + 1) * GL // 16],
                    GL, GL, PAIRS,
                )
                # slice-granular merges keep the tail short and spread the
                # copy work across DVE and ACT under the gather stream
                nc.vector.tensor_copy(mg[:, cs, 0:PAIRS], gx[:, cs, :])
                nc.scalar.copy(mg[:, cs, PAIRS:D], gy[:, cs, :])
                if j == NG // 2 - 1:
                    nc.sync.dma_start(
                        oc[:, 0:HALF, :], mg[:, 0:HALF, :]
                    )
            nc.sync.dma_start(oc[:, HALF:C, :], mg[:, HALF:C, :])

    nc.compile()
    return nc


_cache = {}


def kernel(positions, fixed_table, table_x, table_y):
    nc = _cache.get("nc")
    if nc is None:
        nc = _cache["nc"] = build_nc()
    pos_flat = np.ascontiguousarray(positions.reshape(-1, 2))
    shards = np.split(pos_flat, N_CORES, axis=0)
    fixed_table = np.ascontiguousarray(fixed_table, dtype=np.float32)
    table_x = np.ascontiguousarray(table_x, dtype=np.float32)
    table_y = np.ascontiguousarray(table_y, dtype=np.float32)
    in_maps = [
        {
            "positions": np.ascontiguousarray(s),
            "fixed_table": fixed_table,
            "table_x": table_x,
            "table_y": table_y,
        }
        for s in shards
    ]
    res = run_bass_kernel_spmd(nc, in_maps, core_ids=list(range(N_CORES)))
    outs = [r["out"] for r in res.results]
    return np.concatenate(outs, axis=0).reshape(B, M, R, D)


# revision 13
# speedup vs baseline: 1.2855x; 1.0266x over previous
"""Trainium2 Bass kernel for nn_LookupTableLayer (embedding_lookup).

Full-input contract: kernel(**inputs) takes the full unsharded numpy inputs,
shards positions across 8 NeuronCores (batch dim), runs one SPMD NEFF on
cores 0-7, and returns the full [16, 512, 32, 128] f32 output.

Algorithm:
  reference: t = 0.1*(table/max(table)) + fixed_table; gather rows at idx;
  concat(ex, ey) then reshape(...,128,2).sum(-1) == pair-sum of each gathered
  row. So out[..., 0:64] = pairsum(tx)[idx0], out[..., 64:128] = pairsum(ty)[idx1].
  We precompute the pair-summed 64-wide tables on-chip (1 MB each), store them
  to DRAM scratch, then gather 256 B rows with bulk InstDMAGatherAnt.

  Perf notes (measured on HW):
  - SWDGE descriptor generation on the Pool engine runs at ~8.5 ns/descriptor
    regardless of batching (dma_gather or indirect_dma_start), so the
    2*32768 = 65536 descriptors per core cost ~557 us and dominate. Every
    other engine is scheduled to hide under that stream; the remaining
    levers are the pre-gather ramp and the post-gather tail.
  - dma_gather is capped at 1024 descriptors per instruction (SWDGE ring),
    hence GL=1024 slices.
  - Index prep is per-chunk and pipelined (chunk k+1 prepped during chunk
    k's gathers) so chunk 0's prep is small and off the critical path.

  dma_gather semantics: idx element i of a chunk is read from
  idxs[i%16, i//16] (16-partition wrap, replicated across the 8 gpsimd core
  groups); its gathered row lands at out[i%128, i//128, :]. Positions are
  loaded so slot (p, c) holds token d*1024 + w*64 + c (p = d*16 + w), which
  makes the pos load 512 B-contiguous and the output store 32 KB-contiguous
  per partition.
"""

from contextlib import ExitStack

import numpy as np

import concourse.bacc as bacc
import concourse.bass as bass
import concourse.bass_isa as bass_isa
import concourse.mybir as mybir
import concourse.tile as tile
from concourse.bass_utils import run_bass_kernel_spmd

N_CORES = 8
B, M, R, D = 16, 512, 32, 128
TABLE_LEN = 4106
T = (B // N_CORES) * M * R  # 32768 tokens per core
PAIRS = D // 2  # 64
FLAT_N = TABLE_LEN * D // 128  # 4106 (flat table elems per partition)
PAIR_N = FLAT_N // 2  # 2053
CHUNK = 8192  # tokens per merge/store tile
NCHUNK = T // CHUNK  # 4
C = CHUNK // 128  # 64 gathered tokens per partition per chunk
GL = 1024  # tokens per dma_gather instruction (SWDGE ring = 1024 descs)
NG = CHUNK // GL  # 8 gather slices per chunk
GC = GL // 128  # 8 out columns per gather slice

F32 = mybir.dt.float32
I32 = mybir.dt.int32
I16 = mybir.dt.int16


def _flat(h, p):
    return h[:].rearrange("a b -> (a b)").rearrange("(p n) -> p n", p=p)


def build_nc():
    nc = bacc.Bacc("TRN2", target_bir_lowering=False, debug=False)
    pos = nc.dram_tensor("positions", [T, 2], I32, kind="ExternalInput")
    fixed = nc.dram_tensor("fixed_table", [TABLE_LEN, D], F32, kind="ExternalInput")
    tx = nc.dram_tensor("table_x", [TABLE_LEN, D], F32, kind="ExternalInput")
    ty = nc.dram_tensor("table_y", [TABLE_LEN, D], F32, kind="ExternalInput")
    out = nc.dram_tensor("out", [T, D], F32, kind="ExternalOutput")
    txp_d = nc.dram_tensor("txp", [TABLE_LEN, PAIRS], F32, kind="Internal")
    typ_d = nc.dram_tensor("typ", [TABLE_LEN, PAIRS], F32, kind="Internal")

    with tile.TileContext(nc) as tc, ExitStack() as ctx:
        pwp = ctx.enter_context(tc.tile_pool(name="posw", bufs=2))
        ipp = ctx.enter_context(tc.tile_pool(name="idx", bufs=2))

        def idx_prep(k):
            """Load chunk k's positions in gather-wrap order and split the
            int32 (x, y) pairs into int16 idx tiles via bitcast."""
            posw = pwp.tile([128, 8, C, 2], I32, tag="posw")
            src = pos[k * CHUNK : (k + 1) * CHUNK, :].rearrange(
                "(d w c) j -> w d c j", d=8, w=16, c=C
            )
            for g in range(8):
                nc.sync.dma_start(posw[16 * g : 16 * (g + 1)], src)
            pxk = ipp.tile([128, C, 8], I16, tag="pxk")
            pyk = ipp.tile([128, C, 8], I16, tag="pyk")
            pw16 = posw[:].bitcast(I16)  # [128, 8, C, 4]
            nc.vector.tensor_copy(
                pxk[:].rearrange("p c (d one) -> p c d one", one=1),
                pw16[:, :, :, 0:1].rearrange("p d c one -> p c d one"),
            )
            nc.vector.tensor_copy(
                pyk[:].rearrange("p c (d one) -> p c d one", one=1),
                pw16[:, :, :, 2:3].rearrange("p d c one -> p c d one"),
            )
            return pxk, pyk

        with tc.tile_pool(name="prep", bufs=1) as prep:
            # ---- table preproc: txp = 0.1/max(x)*pairsum(x) + pairsum(fixed)
            # x chain first and tight so the txp store (the first gather's
            # dependency) lands as early as possible.
            xt = prep.tile([128, FLAT_N], F32)
            yt = prep.tile([128, FLAT_N], F32)
            ft = prep.tile([128, FLAT_N], F32)
            nc.sync.dma_start(xt[:], _flat(tx, 128))
            nc.sync.dma_start(ft[:], _flat(fixed, 128))
            nc.sync.dma_start(yt[:], _flat(ty, 128))
            idx0 = idx_prep(0)

            fp = prep.tile([128, PAIR_N], F32)
            fr = ft[:].rearrange("p (n two) -> p n two", two=2)

            def chain(src_t, dram, first):
                mx = prep.tile([128, 1], F32, tag="mx")
                nc.vector.reduce_max(mx[:], src_t[:], axis=mybir.AxisListType.X)
                gm = prep.tile([128, 1], F32, tag="gm")
                nc.gpsimd.partition_all_reduce(gm[:], mx[:], 128, bass_isa.ReduceOp.max)
                sc = prep.tile([128, 1], F32, tag="sc")
                nc.vector.reciprocal(sc[:], gm[:])
                nc.vector.tensor_scalar_mul(sc[:], sc[:], 0.1)
                if first:
                    nc.vector.tensor_add(fp[:], fr[:, :, 0], fr[:, :, 1])
                pr = src_t[:].rearrange("p (n two) -> p n two", two=2)
                ps = prep.tile([128, PAIR_N], F32, tag="ps")
                nc.vector.tensor_add(ps[:], pr[:, :, 0], pr[:, :, 1])
                nc.vector.scalar_tensor_tensor(
                    ps[:], ps[:], sc[:, 0:1], fp[:],
                    op0=mybir.AluOpType.mult, op1=mybir.AluOpType.add,
                )
                nc.sync.dma_start(_flat(dram, 128), ps[:])

            chain(xt, txp_d, True)
            chain(yt, typ_d, False)

        # ---- main loop: sliced gathers/merges, pipelined idx prep,
        # progressively finer stores so the tail after the last gather is
        # only one slice of merge + a small store.
        gp = ctx.enter_context(tc.tile_pool(name="g", bufs=2))
        mp = ctx.enter_context(tc.tile_pool(name="m", bufs=2))
        idx_next = idx0
        for k in range(NCHUNK):
            pxk, pyk = idx_next
            gx = gp.tile([128, C, PAIRS], F32, tag="gx")
            gy = gp.tile([128, C, PAIRS], F32, tag="gy")
            mg = mp.tile([128, C, D], F32, tag="mg")
            idxv_x = pxk[:].rearrange("p c d -> p (c d)")
            idxv_y = pyk[:].rearrange("p c d -> p (c d)")
            oc = out[k * CHUNK : (k + 1) * CHUNK, :].rearrange(
                "(p c) f -> p c f", p=128
            )
            last = k == NCHUNK - 1
            # store boundaries (in gather slices): coarse early, fine late
            bounds = [4, 6, 7, 8] if last else [4, 8]
            done = 0
            for j in range(NG):
                cs = slice(j * GC, (j + 1) * GC)
                nc.gpsimd.dma_gather(
                    gx[:, cs, :], txp_d[:],
                    idxv_x[:, j * GL // 16 : (j + 1) * GL // 16],
                    GL, GL, PAIRS,
                )
                nc.gpsimd.dma_gather(
                    gy[:, cs, :], typ_d[:],
                    idxv_y[:, j * GL // 16 : (j + 1) * GL // 16],
                    GL, GL, PAIRS,
                )
                nc.vector.tensor_copy(mg[:, cs, 0:PAIRS], gx[:, cs, :])
                nc.scalar.copy(mg[:, cs, PAIRS:D], gy[:, cs, :])
                if j == 0 and k + 1 < NCHUNK:
                    idx_next = idx_prep(k + 1)
                if j + 1 in bounds:
                    c0, c1 = done * GC, (j + 1) * GC
                    nc.sync.dma_start(oc[:, c0:c1, :], mg[:, c0:c1, :])
                    done = j + 1

    nc.compile()
    return nc


_cache = {}


def kernel(positions, fixed_table, table_x, table_y):
    nc = _cache.get("nc")
    if nc is None:
        nc = _cache["nc"] = build_nc()
    pos_flat = np.ascontiguousarray(positions.reshape(-1, 2))
    shards = np.split(pos_flat, N_CORES, axis=0)
    fixed_table = np.ascontiguousarray(fixed_table, dtype=np.float32)
    table_x = np.ascontiguousarray(table_x, dtype=np.float32)
    table_y = np.ascontiguousarray(table_y, dtype=np.float32)
    in_maps = [
        {
            "positions": np.ascontiguousarray(s),
            "fixed_table": fixed_table,
            "table_x": table_x,
            "table_y": table_y,
        }
        for s in shards
    ]
    res = run_bass_kernel_spmd(nc, in_maps, core_ids=list(range(N_CORES)))
    outs = [r["out"] for r in res.results]
    return np.concatenate(outs, axis=0).reshape(B, M, R, D)


# revision 15
# speedup vs baseline: 1.2884x; 1.0023x over previous
"""Trainium2 Bass kernel for nn_LookupTableLayer (embedding_lookup).

Full-input contract: kernel(**inputs) takes the full unsharded numpy inputs,
shards positions across 8 NeuronCores (batch dim), runs one SPMD NEFF on
cores 0-7, and returns the full [16, 512, 32, 128] f32 output.

Algorithm:
  reference: t = 0.1*(table/max(table)) + fixed_table; gather rows at idx;
  concat(ex, ey) then reshape(...,128,2).sum(-1) == pair-sum of each gathered
  row. So out[..., 0:64] = pairsum(tx)[idx0], out[..., 64:128] = pairsum(ty)[idx1].
  We precompute the pair-summed 64-wide tables on-chip (1 MB each), store them
  to DRAM scratch, then gather 256 B rows with bulk InstDMAGatherAnt.

  Perf notes (measured on HW):
  - SWDGE descriptor generation on the Pool engine runs at ~8.5 ns/descriptor
    regardless of batching (dma_gather or indirect_dma_start), so the
    2*32768 = 65536 descriptors per core cost ~557 us and dominate. Every
    other engine is scheduled to hide under that stream; the remaining
    levers are the pre-gather ramp and the post-gather tail.
  - dma_gather is capped at 1024 descriptors per instruction (SWDGE ring),
    hence GL=1024 slices.
  - Index prep is per-chunk and pipelined (chunk k+1 prepped during chunk
    k's gathers) so chunk 0's prep is small and off the critical path.

  dma_gather semantics: idx element i of a chunk is read from
  idxs[i%16, i//16] (16-partition wrap, replicated across the 8 gpsimd core
  groups); its gathered row lands at out[i%128, i//128, :]. Positions are
  loaded so slot (p, c) holds token d*1024 + w*64 + c (p = d*16 + w), which
  makes the pos load 512 B-contiguous and the output store 32 KB-contiguous
  per partition.
"""

from contextlib import ExitStack

import numpy as np

import concourse.bacc as bacc
import concourse.bass as bass
import concourse.bass_isa as bass_isa
import concourse.mybir as mybir
import concourse.tile as tile
from concourse.bass_utils import run_bass_kernel_spmd

N_CORES = 8
B, M, R, D = 16, 512, 32, 128
TABLE_LEN = 4106
T = (B // N_CORES) * M * R  # 32768 tokens per core
PAIRS = D // 2  # 64
FLAT_N = TABLE_LEN * D // 128  # 4106 (flat table elems per partition)
PAIR_N = FLAT_N // 2  # 2053
CHUNK = 8192  # tokens per merge/store tile
NCHUNK = T // CHUNK  # 4
C = CHUNK // 128  # 64 gathered tokens per partition per chunk
GL = 1024  # tokens per dma_gather instruction (SWDGE ring = 1024 descs)
NG = CHUNK // GL  # 8 gather slices per chunk
GC = GL // 128  # 8 out columns per gather slice

F32 = mybir.dt.float32
I32 = mybir.dt.int32
I16 = mybir.dt.int16


def _flat(h, p):
    return h[:].rearrange("a b -> (a b)").rearrange("(p n) -> p n", p=p)


def build_nc():
    nc = bacc.Bacc("TRN2", target_bir_lowering=False, debug=False)
    pos = nc.dram_tensor("positions", [T, 2], I32, kind="ExternalInput")
    fixed = nc.dram_tensor("fixed_table", [TABLE_LEN, D], F32, kind="ExternalInput")
    tx = nc.dram_tensor("table_x", [TABLE_LEN, D], F32, kind="ExternalInput")
    ty = nc.dram_tensor("table_y", [TABLE_LEN, D], F32, kind="ExternalInput")
    out = nc.dram_tensor("out", [T, D], F32, kind="ExternalOutput")
    txp_d = nc.dram_tensor("txp", [TABLE_LEN, PAIRS], F32, kind="Internal")
    typ_d = nc.dram_tensor("typ", [TABLE_LEN, PAIRS], F32, kind="Internal")

    with tile.TileContext(nc) as tc, ExitStack() as ctx:
        pwp = ctx.enter_context(tc.tile_pool(name="posw", bufs=2))
        ipp = ctx.enter_context(tc.tile_pool(name="idx", bufs=2))

        def idx_prep(k):
            """Load chunk k's positions in gather-wrap order and split the
            int32 (x, y) pairs into int16 idx tiles via bitcast."""
            posw = pwp.tile([128, 8, C, 2], I32, tag="posw")
            src = pos[k * CHUNK : (k + 1) * CHUNK, :].rearrange(
                "(d w c) j -> w d c j", d=8, w=16, c=C
            )
            for g in range(8):
                nc.sync.dma_start(posw[16 * g : 16 * (g + 1)], src)
            pxk = ipp.tile([128, C, 8], I16, tag="pxk")
            pyk = ipp.tile([128, C, 8], I16, tag="pyk")
            pw16 = posw[:].bitcast(I16)  # [128, 8, C, 4]
            nc.vector.tensor_copy(
                pxk[:].rearrange("p c (d one) -> p c d one", one=1),
                pw16[:, :, :, 0:1].rearrange("p d c one -> p c d one"),
            )
            nc.vector.tensor_copy(
                pyk[:].rearrange("p c (d one) -> p c d one", one=1),
                pw16[:, :, :, 2:3].rearrange("p d c one -> p c d one"),
            )
            return pxk, pyk

        with tc.tile_pool(name="prep", bufs=1) as prep:
            # ---- table preproc: txp = 0.1/max(x)*pairsum(x) + pairsum(fixed)
            # x chain first and tight so the txp store (the first gather's
            # dependency) lands as early as possible.
            xt = prep.tile([128, FLAT_N], F32)
            yt = prep.tile([128, FLAT_N], F32)
            ft = prep.tile([128, FLAT_N], F32)
            nc.sync.dma_start(xt[:], _flat(tx, 128))
            nc.sync.dma_start(ft[:], _flat(fixed, 128))
            nc.sync.dma_start(yt[:], _flat(ty, 128))
            idx0 = idx_prep(0)

            fp = prep.tile([128, PAIR_N], F32)
            fr = ft[:].rearrange("p (n two) -> p n two", two=2)

            def chain(src_t, dram, first):
                # reduce_max -> PAR runs on Pool while the vector engine does
                # the pairsums, so the scalar_tensor_tensor (and the store the
                # first gather waits on) lands as early as possible
                mx = prep.tile([128, 1], F32, tag="mx")
                nc.vector.reduce_max(mx[:], src_t[:], axis=mybir.AxisListType.X)
                gm = prep.tile([128, 1], F32, tag="gm")
                nc.gpsimd.partition_all_reduce(gm[:], mx[:], 128, bass_isa.ReduceOp.max)
                if first:
                    nc.vector.tensor_add(fp[:], fr[:, :, 0], fr[:, :, 1])
                pr = src_t[:].rearrange("p (n two) -> p n two", two=2)
                ps = prep.tile([128, PAIR_N], F32, tag="ps")
                nc.vector.tensor_add(ps[:], pr[:, :, 0], pr[:, :, 1])
                sc = prep.tile([128, 1], F32, tag="sc")
                nc.vector.reciprocal(sc[:], gm[:])
                nc.vector.tensor_scalar_mul(sc[:], sc[:], 0.1)
                nc.vector.scalar_tensor_tensor(
                    ps[:], ps[:], sc[:, 0:1], fp[:],
                    op0=mybir.AluOpType.mult, op1=mybir.AluOpType.add,
                )
                nc.sync.dma_start(_flat(dram, 128), ps[:])

            chain(xt, txp_d, True)
            chain(yt, typ_d, False)

        # ---- main loop: sliced gathers/merges, pipelined idx prep,
        # progressively finer stores so the tail after the last gather is
        # only one slice of merge + a small store.
        gp = ctx.enter_context(tc.tile_pool(name="g", bufs=2))
        mp = ctx.enter_context(tc.tile_pool(name="m", bufs=2))
        idx_next = idx0
        for k in range(NCHUNK):
            pxk, pyk = idx_next
            gx = gp.tile([128, C, PAIRS], F32, tag="gx")
            gy = gp.tile([128, C, PAIRS], F32, tag="gy")
            mg = mp.tile([128, C, D], F32, tag="mg")
            idxv_x = pxk[:].rearrange("p c d -> p (c d)")
            idxv_y = pyk[:].rearrange("p c d -> p (c d)")
            oc = out[k * CHUNK : (k + 1) * CHUNK, :].rearrange(
                "(p c) f -> p c f", p=128
            )
            last = k == NCHUNK - 1
            # all x-slice gathers first: chunk 0's y gathers then start ~70 us
            # in, giving the typ preproc chain slack to finish off-path
            for j in range(NG):
                cs = slice(j * GC, (j + 1) * GC)
                nc.gpsimd.dma_gather(
                    gx[:, cs, :], txp_d[:],
                    idxv_x[:, j * GL // 16 : (j + 1) * GL // 16],
                    GL, GL, PAIRS,
                )
                nc.vector.tensor_copy(mg[:, cs, 0:PAIRS], gx[:, cs, :])
                if j == 0 and k + 1 < NCHUNK:
                    idx_next = idx_prep(k + 1)
            # store boundaries (in y gather slices): coarse early, fine late
            bounds = [4, 6, 7, 8] if last else [4, 8]
            done = 0
            for j in range(NG):
                cs = slice(j * GC, (j + 1) * GC)
                nc.gpsimd.dma_gather(
                    gy[:, cs, :], typ_d[:],
                    idxv_y[:, j * GL // 16 : (j + 1) * GL // 16],
                    GL, GL, PAIRS,
                )
                nc.scalar.copy(mg[:, cs, PAIRS:D], gy[:, cs, :])
                if j + 1 in bounds:
                    c0, c1 = done * GC, (j + 1) * GC
                    nc.sync.dma_start(oc[:, c0:c1, :], mg[:, c0:c1, :])
                    done = j + 1

    nc.compile()
    return nc


_cache = {}


def kernel(positions, fixed_table, table_x, table_y):
    nc = _cache.get("nc")
    if nc is None:
        nc = _cache["nc"] = build_nc()
    pos_flat = np.ascontiguousarray(positions.reshape(-1, 2))
    shards = np.split(pos_flat, N_CORES, axis=0)
    fixed_table = np.ascontiguousarray(fixed_table, dtype=np.float32)
    table_x = np.ascontiguousarray(table_x, dtype=np.float32)
    table_y = np.ascontiguousarray(table_y, dtype=np.float32)
    in_maps = [
        {
            "positions": np.ascontiguousarray(s),
            "fixed_table": fixed_table,
            "table_x": table_x,
            "table_y": table_y,
        }
        for s in shards
    ]
    res = run_bass_kernel_spmd(nc, in_maps, core_ids=list(range(N_CORES)))
    outs = [r["out"] for r in res.results]
    return np.concatenate(outs, axis=0).reshape(B, M, R, D)


# revision 16
# speedup vs baseline: 1.2903x; 1.0014x over previous
"""Trainium2 Bass kernel for nn_LookupTableLayer (embedding_lookup).

Full-input contract: kernel(**inputs) takes the full unsharded numpy inputs,
shards positions across 8 NeuronCores (batch dim), runs one SPMD NEFF on
cores 0-7, and returns the full [16, 512, 32, 128] f32 output.

Algorithm:
  reference: t = 0.1*(table/max(table)) + fixed_table; gather rows at idx;
  concat(ex, ey) then reshape(...,128,2).sum(-1) == pair-sum of each gathered
  row. So out[..., 0:64] = pairsum(tx)[idx0], out[..., 64:128] = pairsum(ty)[idx1].
  We precompute the pair-summed 64-wide tables on-chip (1 MB each), store them
  to DRAM scratch, then gather 256 B rows with bulk InstDMAGatherAnt.

  Perf notes (measured on HW):
  - SWDGE descriptor generation on the Pool engine runs at ~8.5 ns/descriptor
    regardless of batching (dma_gather or indirect_dma_start), so the
    2*32768 = 65536 descriptors per core cost ~557 us and dominate. Every
    other engine is scheduled to hide under that stream; the remaining
    levers are the pre-gather ramp and the post-gather tail.
  - dma_gather is capped at 1024 descriptors per instruction (SWDGE ring),
    hence GL=1024 slices.
  - Index prep is per-chunk and pipelined (chunk k+1 prepped during chunk
    k's gathers) so chunk 0's prep is small and off the critical path.

  dma_gather semantics: idx element i of a chunk is read from
  idxs[i%16, i//16] (16-partition wrap, replicated across the 8 gpsimd core
  groups); its gathered row lands at out[i%128, i//128, :]. Positions are
  loaded so slot (p, c) holds token d*1024 + w*64 + c (p = d*16 + w), which
  makes the pos load 512 B-contiguous and the output store 32 KB-contiguous
  per partition.
"""

from contextlib import ExitStack

import numpy as np

import concourse.bacc as bacc
import concourse.bass as bass
import concourse.bass_isa as bass_isa
import concourse.mybir as mybir
import concourse.tile as tile
from concourse.bass_utils import run_bass_kernel_spmd

N_CORES = 8
B, M, R, D = 16, 512, 32, 128
TABLE_LEN = 4106
T = (B // N_CORES) * M * R  # 32768 tokens per core
PAIRS = D // 2  # 64
FLAT_N = TABLE_LEN * D // 128  # 4106 (flat table elems per partition)
PAIR_N = FLAT_N // 2  # 2053
CHUNK = 8192  # tokens per merge/store tile
NCHUNK = T // CHUNK  # 4
C = CHUNK // 128  # 64 gathered tokens per partition per chunk
GL = 1024  # tokens per dma_gather instruction (SWDGE ring = 1024 descs)
NG = CHUNK // GL  # 8 gather slices per chunk
GC = GL // 128  # 8 out columns per gather slice

F32 = mybir.dt.float32
I32 = mybir.dt.int32
I16 = mybir.dt.int16


def _flat(h, p):
    return h[:].rearrange("a b -> (a b)").rearrange("(p n) -> p n", p=p)


def build_nc():
    nc = bacc.Bacc("TRN2", target_bir_lowering=False, debug=False)
    pos = nc.dram_tensor("positions", [T, 2], I32, kind="ExternalInput")
    fixed = nc.dram_tensor("fixed_table", [TABLE_LEN, D], F32, kind="ExternalInput")
    tx = nc.dram_tensor("table_x", [TABLE_LEN, D], F32, kind="ExternalInput")
    ty = nc.dram_tensor("table_y", [TABLE_LEN, D], F32, kind="ExternalInput")
    out = nc.dram_tensor("out", [T, D], F32, kind="ExternalOutput")
    txp_d = nc.dram_tensor("txp", [TABLE_LEN, PAIRS], F32, kind="Internal")
    typ_d = nc.dram_tensor("typ", [TABLE_LEN, PAIRS], F32, kind="Internal")

    with tile.TileContext(nc) as tc, ExitStack() as ctx:
        pwp = ctx.enter_context(tc.tile_pool(name="posw", bufs=2))
        ipp = ctx.enter_context(tc.tile_pool(name="idx", bufs=2))

        def idx_prep(k):
            """Load chunk k's positions in gather-wrap order and split the
            int32 (x, y) pairs into int16 idx tiles via bitcast."""
            posw = pwp.tile([128, 8, C, 2], I32, tag="posw")
            src = pos[k * CHUNK : (k + 1) * CHUNK, :].rearrange(
                "(d w c) j -> w d c j", d=8, w=16, c=C
            )
            for g in range(8):
                nc.sync.dma_start(posw[16 * g : 16 * (g + 1)], src)
            pxk = ipp.tile([128, C, 8], I16, tag="pxk")
            pyk = ipp.tile([128, C, 8], I16, tag="pyk")
            pw16 = posw[:].bitcast(I16)  # [128, 8, C, 4]
            nc.vector.tensor_copy(
                pxk[:].rearrange("p c (d one) -> p c d one", one=1),
                pw16[:, :, :, 0:1].rearrange("p d c one -> p c d one"),
            )
            nc.vector.tensor_copy(
                pyk[:].rearrange("p c (d one) -> p c d one", one=1),
                pw16[:, :, :, 2:3].rearrange("p d c one -> p c d one"),
            )
            return pxk, pyk

        with tc.tile_pool(name="prep", bufs=1) as prep:
            # ---- table preproc: txp = 0.1/max(x)*pairsum(x) + pairsum(fixed)
            # x chain first and tight so the txp store (the first gather's
            # dependency) lands as early as possible.
            xt = prep.tile([128, FLAT_N], F32)
            yt = prep.tile([128, FLAT_N], F32)
            ft = prep.tile([128, FLAT_N], F32)
            nc.sync.dma_start(xt[:], _flat(tx, 128))
            nc.sync.dma_start(ft[:], _flat(fixed, 128))
            nc.sync.dma_start(yt[:], _flat(ty, 128))
            idx0 = idx_prep(0)

            fp = prep.tile([128, PAIR_N], F32)
            fr = ft[:].rearrange("p (n two) -> p n two", two=2)

            def chain(src_t, dram, first):
                # reduce_max -> PAR runs on Pool while the vector engine does
                # the pairsums, so the scalar_tensor_tensor (and the store the
                # first gather waits on) lands as early as possible
                mx = prep.tile([128, 1], F32, tag="mx")
                nc.vector.reduce_max(mx[:], src_t[:], axis=mybir.AxisListType.X)
                gm = prep.tile([128, 1], F32, tag="gm")
                nc.gpsimd.partition_all_reduce(gm[:], mx[:], 128, bass_isa.ReduceOp.max)
                if first:
                    nc.vector.tensor_add(fp[:], fr[:, :, 0], fr[:, :, 1])
                pr = src_t[:].rearrange("p (n two) -> p n two", two=2)
                ps = prep.tile([128, PAIR_N], F32, tag="ps")
                nc.vector.tensor_add(ps[:], pr[:, :, 0], pr[:, :, 1])
                sc = prep.tile([128, 1], F32, tag="sc")
                nc.vector.reciprocal(sc[:], gm[:])
                nc.vector.tensor_scalar_mul(sc[:], sc[:], 0.1)
                nc.vector.scalar_tensor_tensor(
                    ps[:], ps[:], sc[:, 0:1], fp[:],
                    op0=mybir.AluOpType.mult, op1=mybir.AluOpType.add,
                )
                nc.sync.dma_start(_flat(dram, 128), ps[:])

            chain(xt, txp_d, True)
            chain(yt, typ_d, False)

        # ---- main loop: sliced gathers/merges, pipelined idx prep,
        # progressively finer stores so the tail after the last gather is
        # only one slice of merge + a small store.
        gp = ctx.enter_context(tc.tile_pool(name="g", bufs=2))
        mp = ctx.enter_context(tc.tile_pool(name="m", bufs=2))
        idx_next = idx0
        for k in range(NCHUNK):
            pxk, pyk = idx_next
            gx = gp.tile([128, C, PAIRS], F32, tag="gx")
            gy = gp.tile([128, C, PAIRS], F32, tag="gy")
            mg = mp.tile([128, C, D], F32, tag="mg")
            idxv_x = pxk[:].rearrange("p c d -> p (c d)")
            idxv_y = pyk[:].rearrange("p c d -> p (c d)")
            oc = out[k * CHUNK : (k + 1) * CHUNK, :].rearrange(
                "(p c) f -> p c f", p=128
            )
            last = k == NCHUNK - 1
            # all x-slice gathers first: chunk 0's y gathers then start ~70 us
            # in, giving the typ preproc chain slack to finish off-path
            for j in range(NG):
                cs = slice(j * GC, (j + 1) * GC)
                nc.gpsimd.dma_gather(
                    gx[:, cs, :], txp_d[:],
                    idxv_x[:, j * GL // 16 : (j + 1) * GL // 16],
                    GL, GL, PAIRS,
                )
                nc.vector.tensor_copy(mg[:, cs, 0:PAIRS], gx[:, cs, :])
                # prefetch next chunk's indices mid-chunk: at j==0 the DMA
                # burst contends with the first gather's ring drain
                if j == 4 and k + 1 < NCHUNK:
                    idx_next = idx_prep(k + 1)
            # store boundaries (in y gather slices): coarse early, fine late
            bounds = [4, 6, 7, 8] if last else [4, 8]
            done = 0
            for j in range(NG):
                cs = slice(j * GC, (j + 1) * GC)
                nc.gpsimd.dma_gather(
                    gy[:, cs, :], typ_d[:],
                    idxv_y[:, j * GL // 16 : (j + 1) * GL // 16],
                    GL, GL, PAIRS,
                )
                nc.scalar.copy(mg[:, cs, PAIRS:D], gy[:, cs, :])
                if j + 1 in bounds:
                    c0, c1 = done * GC, (j + 1) * GC
                    nc.sync.dma_start(oc[:, c0:c1, :], mg[:, c0:c1, :])
                    done = j + 1

    nc.compile()
    return nc


_cache = {}


def kernel(positions, fixed_table, table_x, table_y):
    nc = _cache.get("nc")
    if nc is None:
        nc = _cache["nc"] = build_nc()
    pos_flat = np.ascontiguousarray(positions.reshape(-1, 2))
    shards = np.split(pos_flat, N_CORES, axis=0)
    fixed_table = np.ascontiguousarray(fixed_table, dtype=np.float32)
    table_x = np.ascontiguousarray(table_x, dtype=np.float32)
    table_y = np.ascontiguousarray(table_y, dtype=np.float32)
    in_maps = [
        {
            "positions": np.ascontiguousarray(s),
            "fixed_table": fixed_table,
            "table_x": table_x,
            "table_y": table_y,
        }
        for s in shards
    ]
    res = run_bass_kernel_spmd(nc, in_maps, core_ids=list(range(N_CORES)))
    outs = [r["out"] for r in res.results]
    return np.concatenate(outs, axis=0).reshape(B, M, R, D)


# revision 18
# speedup vs baseline: 1.3053x; 1.0116x over previous
"""Trainium2 Bass kernel for nn_LookupTableLayer (embedding_lookup).

Full-input contract: kernel(**inputs) takes the full unsharded numpy inputs,
shards positions across 8 NeuronCores (batch dim), runs one SPMD NEFF on
cores 0-7, and returns the full [16, 512, 32, 128] f32 output.

Algorithm:
  reference: t = 0.1*(table/max(table)) + fixed_table; gather rows at idx;
  concat(ex, ey) then reshape(...,128,2).sum(-1) == pair-sum of each gathered
  row. So out[..., 0:64] = pairsum(tx)[idx0], out[..., 64:128] = pairsum(ty)[idx1].
  We precompute the pair-summed 64-wide tables on-chip (1 MB each), store them
  to DRAM scratch, then gather 256 B rows with bulk InstDMAGatherAnt.

  Perf notes (measured on HW):
  - SWDGE descriptor generation on the Pool engine runs at ~8.5 ns/descriptor
    regardless of batching (dma_gather or indirect_dma_start), so the
    2*32768 = 65536 descriptors per core cost ~557 us and dominate. Every
    other engine is scheduled to hide under that stream; the remaining
    levers are the pre-gather ramp and the post-gather tail.
  - dma_gather is capped at 1024 descriptors per instruction (SWDGE ring),
    hence GL=1024 slices.
  - Index prep is per-chunk and pipelined (chunk k+1 prepped during chunk
    k's gathers) so chunk 0's prep is small and off the critical path.

  dma_gather semantics: idx element i of a chunk is read from
  idxs[i%16, i//16] (16-partition wrap, replicated across the 8 gpsimd core
  groups); its gathered row lands at out[i%128, i//128, :]. Positions are
  loaded so slot (p, c) holds token d*1024 + w*64 + c (p = d*16 + w), which
  makes the pos load 512 B-contiguous and the output store 32 KB-contiguous
  per partition.
"""

from contextlib import ExitStack

import numpy as np

import concourse.bacc as bacc
import concourse.bass as bass
import concourse.bass_isa as bass_isa
import concourse.mybir as mybir
import concourse.tile as tile
from concourse.bass_utils import run_bass_kernel_spmd

N_CORES = 8
B, M, R, D = 16, 512, 32, 128
TABLE_LEN = 4106
T = (B // N_CORES) * M * R  # 32768 tokens per core
PAIRS = D // 2  # 64
FLAT_N = TABLE_LEN * D // 128  # 4106 (flat table elems per partition)
PAIR_N = FLAT_N // 2  # 2053
CHUNK = 8192  # tokens per merge/store tile
NCHUNK = T // CHUNK  # 4
C = CHUNK // 128  # 64 gathered tokens per partition per chunk
GL = 1024  # tokens per dma_gather instruction (SWDGE ring = 1024 descs)
NG = CHUNK // GL  # 8 gather slices per chunk
GC = GL // 128  # 8 out columns per gather slice

F32 = mybir.dt.float32
I32 = mybir.dt.int32
I16 = mybir.dt.int16


def _flat(h, p):
    return h[:].rearrange("a b -> (a b)").rearrange("(p n) -> p n", p=p)


def build_nc():
    nc = bacc.Bacc("TRN2", target_bir_lowering=False, debug=False)
    pos = nc.dram_tensor("positions", [T, 2], I32, kind="ExternalInput")
    fixed = nc.dram_tensor("fixed_table", [TABLE_LEN, D], F32, kind="ExternalInput")
    tx = nc.dram_tensor("table_x", [TABLE_LEN, D], F32, kind="ExternalInput")
    ty = nc.dram_tensor("table_y", [TABLE_LEN, D], F32, kind="ExternalInput")
    out = nc.dram_tensor("out", [T, D], F32, kind="ExternalOutput")
    txp_d = nc.dram_tensor("txp", [TABLE_LEN, PAIRS], F32, kind="Internal")
    typ_d = nc.dram_tensor("typ", [TABLE_LEN, PAIRS], F32, kind="Internal")
    warm_d = nc.dram_tensor("warm", [128, PAIRS], F32, kind="Internal")

    with tile.TileContext(nc) as tc, ExitStack() as ctx:
        pwp = ctx.enter_context(tc.tile_pool(name="posw", bufs=2))
        ipp = ctx.enter_context(tc.tile_pool(name="idx", bufs=2))

        def idx_prep(k):
            """Load chunk k's positions in gather-wrap order and split the
            int32 (x, y) pairs into int16 idx tiles via bitcast."""
            posw = pwp.tile([128, 8, C, 2], I32, tag="posw")
            src = pos[k * CHUNK : (k + 1) * CHUNK, :].rearrange(
                "(d w c) j -> w d c j", d=8, w=16, c=C
            )
            for g in range(8):
                nc.sync.dma_start(posw[16 * g : 16 * (g + 1)], src)
            pxk = ipp.tile([128, C, 8], I16, tag="pxk")
            pyk = ipp.tile([128, C, 8], I16, tag="pyk")
            pw16 = posw[:].bitcast(I16)  # [128, 8, C, 4]
            nc.vector.tensor_copy(
                pxk[:].rearrange("p c (d one) -> p c d one", one=1),
                pw16[:, :, :, 0:1].rearrange("p d c one -> p c d one"),
            )
            nc.vector.tensor_copy(
                pyk[:].rearrange("p c (d one) -> p c d one", one=1),
                pw16[:, :, :, 2:3].rearrange("p d c one -> p c d one"),
            )
            return pxk, pyk

        with tc.tile_pool(name="prep", bufs=1) as prep:
            # ---- table preproc: txp = 0.1/max(x)*pairsum(x) + pairsum(fixed)
            # x chain first and tight so the txp store (the first gather's
            # dependency) lands as early as possible.
            xt = prep.tile([128, FLAT_N], F32)
            yt = prep.tile([128, FLAT_N], F32)
            ft = prep.tile([128, FLAT_N], F32)
            nc.sync.dma_start(xt[:], _flat(tx, 128))
            nc.sync.dma_start(ft[:], _flat(fixed, 128))
            nc.sync.dma_start(yt[:], _flat(ty, 128))
            idx0 = idx_prep(0)

            fp = prep.tile([128, PAIR_N], F32)
            fr = ft[:].rearrange("p (n two) -> p n two", two=2)

            def chain(src_t, dram, first):
                # reduce_max -> PAR runs on Pool while the vector engine does
                # the pairsums, so the scalar_tensor_tensor (and the store the
                # first gather waits on) lands as early as possible
                mx = prep.tile([128, 1], F32, tag="mx")
                nc.vector.reduce_max(mx[:], src_t[:], axis=mybir.AxisListType.X)
                gm = prep.tile([128, 1], F32, tag="gm")
                nc.gpsimd.partition_all_reduce(gm[:], mx[:], 128, bass_isa.ReduceOp.max)
                if first:
                    nc.vector.tensor_add(fp[:], fr[:, :, 0], fr[:, :, 1])
                pr = src_t[:].rearrange("p (n two) -> p n two", two=2)
                ps = prep.tile([128, PAIR_N], F32, tag="ps")
                nc.vector.tensor_add(ps[:], pr[:, :, 0], pr[:, :, 1])
                sc = prep.tile([128, 1], F32, tag="sc")
                nc.vector.reciprocal(sc[:], gm[:])
                nc.vector.tensor_scalar_mul(sc[:], sc[:], 0.1)
                nc.vector.scalar_tensor_tensor(
                    ps[:], ps[:], sc[:, 0:1], fp[:],
                    op0=mybir.AluOpType.mult, op1=mybir.AluOpType.add,
                )
                nc.sync.dma_start(_flat(dram, 128), ps[:])

            chain(xt, txp_d, True)
            # warmup gather on a dummy table during the ramp: the first real
            # dma_gather otherwise pays ~15 us of cold ucode/ring state
            warm_idx = prep.tile([128, 8], I16, tag="warm_idx")
            nc.gpsimd.memset(warm_idx[:], 0)
            warm_g = prep.tile([128, 1, PAIRS], F32, tag="warm_g")
            nc.gpsimd.dma_gather(warm_g[:], warm_d[:], warm_idx[:], 128, 128, PAIRS)
            chain(yt, typ_d, False)

        # ---- main loop: sliced gathers/merges, pipelined idx prep,
        # progressively finer stores so the tail after the last gather is
        # only one slice of merge + a small store.
        gp = ctx.enter_context(tc.tile_pool(name="g", bufs=2))
        mp = ctx.enter_context(tc.tile_pool(name="m", bufs=2))
        idx_next = idx0
        for k in range(NCHUNK):
            pxk, pyk = idx_next
            gx = gp.tile([128, C, PAIRS], F32, tag="gx")
            gy = gp.tile([128, C, PAIRS], F32, tag="gy")
            mg = mp.tile([128, C, D], F32, tag="mg")
            idxv_x = pxk[:].rearrange("p c d -> p (c d)")
            idxv_y = pyk[:].rearrange("p c d -> p (c d)")
            oc = out[k * CHUNK : (k + 1) * CHUNK, :].rearrange(
                "(p c) f -> p c f", p=128
            )
            last = k == NCHUNK - 1
            # all x-slice gathers first: chunk 0's y gathers then start ~70 us
            # in, giving the typ preproc chain slack to finish off-path
            for j in range(NG):
                cs = slice(j * GC, (j + 1) * GC)
                nc.gpsimd.dma_gather(
                    gx[:, cs, :], txp_d[:],
                    idxv_x[:, j * GL // 16 : (j + 1) * GL // 16],
                    GL, GL, PAIRS,
                )
                nc.vector.tensor_copy(mg[:, cs, 0:PAIRS], gx[:, cs, :])
                # prefetch next chunk's indices mid-chunk: at j==0 the DMA
                # burst contends with the first gather's ring drain
                if j == 4 and k + 1 < NCHUNK:
                    idx_next = idx_prep(k + 1)
            # store boundaries (in y gather slices): coarse early, fine late
            bounds = [4, 6, 7, 8] if last else [4, 8]
            done = 0
            for j in range(NG):
                cs = slice(j * GC, (j + 1) * GC)
                nc.gpsimd.dma_gather(
                    gy[:, cs, :], typ_d[:],
                    idxv_y[:, j * GL // 16 : (j + 1) * GL // 16],
                    GL, GL, PAIRS,
                )
                nc.scalar.copy(mg[:, cs, PAIRS:D], gy[:, cs, :])
                if j + 1 in bounds:
                    c0, c1 = done * GC, (j + 1) * GC
                    nc.sync.dma_start(oc[:, c0:c1, :], mg[:, c0:c1, :])
                    done = j + 1

    nc.compile()
    return nc


_cache = {}


def kernel(positions, fixed_table, table_x, table_y):
    nc = _cache.get("nc")
    if nc is None:
        nc = _cache["nc"] = build_nc()
    pos_flat = np.ascontiguousarray(positions.reshape(-1, 2))
    shards = np.split(pos_flat, N_CORES, axis=0)
    fixed_table = np.ascontiguousarray(fixed_table, dtype=np.float32)
    table_x = np.ascontiguousarray(table_x, dtype=np.float32)
    table_y = np.ascontiguousarray(table_y, dtype=np.float32)
    in_maps = [
        {
            "positions": np.ascontiguousarray(s),
            "fixed_table": fixed_table,
            "table_x": table_x,
            "table_y": table_y,
        }
        for s in shards
    ]
    res = run_bass_kernel_spmd(nc, in_maps, core_ids=list(range(N_CORES)))
    outs = [r["out"] for r in res.results]
    return np.concatenate(outs, axis=0).reshape(B, M, R, D)
